# revision 12
# baseline (speedup 1.0000x reference)
"""GRU decoder kernel for Trainium2, 8-core data-parallel.

Problem: h_t recurrence over 512 steps, B=256, H=512 (3 gates).
  xr = relu(x); gi = xr @ W_ih.T + b_ih   (constant over time)
  per step: gh = h @ W_hh.T + b_hh
            r = sigmoid(gi_r + gh_r); z = sigmoid(gi_z + gh_z)
            n = tanh(gi_n + r * gh_n); h = (1-z)*n + z*h
Output: [B, S, H] stacked h_t.

Sharding: pure data parallel, batch/8 = 32 rows per core. Weights replicated.

Per-core design (mapping: psum[batch, gate_out], stationary = h^T tiles):
  - 3 gates computed concurrently via PE col-tiling at tile_position (0,0)/(0,32)/(0,64)
  - gi_r/gi_z and b_hh_n injected into PSUM with identity/ones stationary matmuls
    (b_ih+b_hh folded into gi for r/z on host)
  - hidden dim split in NCHUNK free-dim chunks so the elementwise chain of
    chunk c overlaps the matmuls of chunk c+1 (and PE stays HAM-warm)
  - bf16 state everywhere (DVE 2x mode); f32 only in PSUM and the DRAM output
  - h^T maintained via per-chunk PE transposes (bf16)
"""

import numpy as np

import concourse.bass as bass
from concourse import bacc
import concourse.mybir as mybir
from concourse.tile import TileContext
from concourse.bass_utils import run_bass_kernel_spmd
from concourse.masks import make_identity

B, H, SEQ, NCORES = 256, 512, 512, 8
BL = B // NCORES  # 32 batch rows per core
F32 = mybir.dt.float32
FP16 = mybir.dt.float16
AF = mybir.ActivationFunctionType
NCHUNK = 2
CK = H // NCHUNK  # chunk width in the hidden/free dim


def build_nc(seq_len=SEQ):
    nc = bacc.Bacc("TRN2")
    x_p = nc.declare_dram_parameter("x", [BL, H], F32, isOutput=False)
    wih_p = nc.declare_dram_parameter("W_ihT", [H, 3 * H], F32, isOutput=False)
    whh_p = nc.declare_dram_parameter("W_hhT", [H, 3 * H], F32, isOutput=False)
    bc_p = nc.declare_dram_parameter("b_comb", [1, 3 * H], F32, isOutput=False)
    bn_p = nc.declare_dram_parameter("b_hh_n", [1, H], F32, isOutput=False)
    out_p = nc.declare_dram_parameter("out", [BL, seq_len, H], F32, isOutput=True)

    with TileContext(nc) as tc:
        with (
            tc.tile_pool(name="const", bufs=1) as cpool,
            tc.tile_pool(name="wpool", bufs=1) as wpool,
            tc.tile_pool(name="work", bufs=4) as pool,
            tc.tile_pool(name="hpool", bufs=3) as hpool,
            tc.tile_pool(name="pgates", bufs=2, space="PSUM") as ppool,
            tc.tile_pool(name="ptrans", bufs=2, space="PSUM") as tpool,
        ):
            # ---- constants (bf16 matmul operands) ----
            ident_f = cpool.tile([32, 32], F32)
            make_identity(nc, ident_f)
            ident = cpool.tile([32, 32], FP16)
            nc.vector.tensor_copy(out=ident, in_=ident_f)
            ones1 = cpool.tile([1, 32], FP16)
            nc.vector.memset(ones1, 1.0)
            bhhn_f = cpool.tile([1, H], F32)
            nc.sync.dma_start(out=bhhn_f, in_=bn_p[:, :])
            bhhn = cpool.tile([1, H], FP16)
            nc.vector.tensor_copy(out=bhhn, in_=bhhn_f)
            bcomb_f = cpool.tile([1, 3 * H], F32)
            nc.sync.dma_start(out=bcomb_f, in_=bc_p[:, :])
            bcomb = cpool.tile([1, 3 * H], FP16)
            nc.vector.tensor_copy(out=bcomb, in_=bcomb_f)

            # ---- weights: k-tiles [128, 1536] in bf16 ----
            whh_sb = []
            wih_sb = []
            for j in range(4):
                tf = wpool.tile([128, 3 * H], F32, tag="wstage")
                nc.sync.dma_start(out=tf, in_=whh_p[128 * j : 128 * (j + 1), :])
                tb = wpool.tile([128, 3 * H], FP16, tag=f"whhb{j}")
                nc.vector.tensor_copy(out=tb, in_=tf)
                whh_sb.append(tb)
            for j in range(4):
                tf = wpool.tile([128, 3 * H], F32, tag="wstage")
                nc.sync.dma_start(out=tf, in_=wih_p[128 * j : 128 * (j + 1), :])
                tb = wpool.tile([128, 3 * H], FP16, tag=f"wihb{j}")
                nc.vector.tensor_copy(out=tb, in_=tf)
                wih_sb.append(tb)

            # ---- x -> relu(bf16) -> transpose ----
            x_sb = pool.tile([BL, H], F32, tag="x")
            nc.sync.dma_start(out=x_sb, in_=x_p[:, :])
            xr = pool.tile([BL, H], FP16, tag="xr")
            nc.scalar.activation(out=xr, in_=x_sb, func=AF.Relu)
            psT0 = tpool.tile([128, 128], FP16, tag="pT")
            for j in range(4):
                nc.tensor.transpose(
                    out=psT0[:, 32 * j : 32 * (j + 1)],
                    in_=xr[:, 128 * j : 128 * (j + 1)],
                    identity=ident,
                )
            xrT = pool.tile([128, 128], FP16, tag="xrT")
            nc.vector.tensor_copy(out=xrT, in_=psT0)

            # ---- gi = xr @ W_ihT + b_comb  -> sbuf bf16 [32, 1536] ----
            gi = pool.tile([BL, 3 * H], FP16, tag="gi")
            for g in range(3):
                pg = ppool.tile([128, 512], F32, tag="ps")
                for j in range(4):
                    nc.tensor.matmul(
                        out=pg[0:BL, :],
                        lhsT=xrT[:, 32 * j : 32 * (j + 1)],
                        rhs=wih_sb[j][:, 512 * g : 512 * (g + 1)],
                        start=(j == 0),
                        stop=False,
                    )
                nc.tensor.matmul(
                    out=pg[0:BL, :],
                    lhsT=ones1,
                    rhs=bcomb[:, 512 * g : 512 * (g + 1)],
                    start=False,
                    stop=True,
                )
                nc.scalar.copy(out=gi[:, 512 * g : 512 * (g + 1)], in_=pg[0:BL, :])

            # ---- initial state (h == 0) ----
            hT = pool.tile([128, 128], FP16, tag="hT")
            nc.vector.memset(hT, 0.0)
            h = hpool.tile([BL, H], F32, tag="h")
            nc.vector.memset(h, 0.0)

            # ---- recurrence ----
            for t in range(seq_len):
                ps = ppool.tile([128, 512], F32, tag="ps")
                psT = tpool.tile([128, 128], F32, tag="pT")
                hT_new = pool.tile([128, 128], FP16, tag="hT")
                h_new = hpool.tile([BL, H], F32, tag="h")
                for c in range(NCHUNK):
                    cs = slice(CK * c, CK * (c + 1))
                    # --- matmuls for this free-dim chunk (3 gates col-tiled) ---
                    nc.tensor.matmul(
                        out=ps[0:32, cs], lhsT=ident, rhs=gi[:, CK * c : CK * (c + 1)],
                        start=True, stop=False, tile_position=(0, 0),
                    )
                    nc.tensor.matmul(
                        out=ps[32:64, cs], lhsT=ident,
                        rhs=gi[:, 512 + CK * c : 512 + CK * (c + 1)],
                        start=True, stop=False, tile_position=(0, 32),
                    )
                    nc.tensor.matmul(
                        out=ps[64:96, cs], lhsT=ones1, rhs=bhhn[:, cs],
                        start=True, stop=False, tile_position=(0, 64),
                    )
                    for j in range(4):
                        for g in range(3):
                            nc.tensor.matmul(
                                out=ps[32 * g : 32 * (g + 1), cs],
                                lhsT=hT[:, 32 * j : 32 * (j + 1)],
                                rhs=whh_sb[j][:, 512 * g + CK * c : 512 * g + CK * (c + 1)],
                                start=False,
                                stop=(j == 3),
                                tile_position=(0, 32 * g),
                            )
                    # --- gates for this chunk ---
                    r = pool.tile([32, CK], FP16, tag="r")
                    nc.scalar.activation(out=r, in_=ps[0:32, cs], func=AF.Sigmoid)
                    z = pool.tile([32, CK], FP16, tag="z")
                    nc.scalar.activation(out=z, in_=ps[32:64, cs], func=AF.Sigmoid)
                    m = pool.tile([32, CK], FP16, tag="m")
                    nc.vector.tensor_mul(out=m, in0=r, in1=ps[64:96, cs])
                    a = pool.tile([32, CK], FP16, tag="a")
                    nc.vector.tensor_add(out=a, in0=m, in1=gi[:, 1024 + CK * c : 1024 + CK * (c + 1)])
                    n_ = pool.tile([32, CK], FP16, tag="n")
                    nc.scalar.activation(out=n_, in_=a, func=AF.Tanh)
                    # h_new = n + z*(h - n); h kept f32 (it is the DRAM output)
                    d = pool.tile([32, CK], FP16, tag="d")
                    nc.gpsimd.tensor_sub(out=d, in0=h[:, cs], in1=n_)
                    t2 = pool.tile([32, CK], FP16, tag="t2")
                    nc.vector.tensor_mul(out=t2, in0=z, in1=d)
                    nc.vector.tensor_add(out=h_new[:, cs], in0=n_, in1=t2)
                    # transpose this chunk for the next step's stationaries
                    if t < seq_len - 1:
                        for jj in range(2 * c, 2 * c + 2):
                            nc.tensor.transpose(
                                out=psT[:, 32 * jj : 32 * (jj + 1)],
                                in_=h_new[:, 128 * jj : 128 * (jj + 1)],
                                identity=ident_f,
                            )
                        nc.vector.tensor_copy(
                            out=hT_new[:, 64 * c : 64 * (c + 1)],
                            in_=psT[:, 64 * c : 64 * (c + 1)],
                        )
                nc.sync.dma_start(out=out_p[:, t, :], in_=h_new)
                h = h_new
                hT = hT_new
    nc.finalize()
    return nc


def kernel(x, W_ih, W_hh, b_ih, b_hh, seq_len):
    x = np.asarray(x, dtype=np.float32)
    W_ih = np.asarray(W_ih, dtype=np.float32)
    W_hh = np.asarray(W_hh, dtype=np.float32)
    b_ih = np.asarray(b_ih, dtype=np.float32)
    b_hh = np.asarray(b_hh, dtype=np.float32)
    seq_len = int(seq_len)

    W_ihT = np.ascontiguousarray(W_ih.T)
    W_hhT = np.ascontiguousarray(W_hh.T)
    b_comb = np.concatenate([b_ih[: 2 * H] + b_hh[: 2 * H], b_ih[2 * H :]]).reshape(1, -1)
    b_hh_n = np.ascontiguousarray(b_hh[2 * H :]).reshape(1, -1)

    nc = build_nc(seq_len=seq_len)
    in_maps = [
        {
            "x": np.ascontiguousarray(x[BL * i : BL * (i + 1)]),
            "W_ihT": W_ihT,
            "W_hhT": W_hhT,
            "b_comb": b_comb,
            "b_hh_n": b_hh_n,
        }
        for i in range(NCORES)
    ]
    res = run_bass_kernel_spmd(nc, in_maps, core_ids=list(range(NCORES)))
    outs = [np.asarray(res.results[i]["out"], dtype=np.float32) for i in range(NCORES)]
    return np.concatenate(outs, axis=0)


if __name__ == "__main__":
    rng = np.random.default_rng(0)
    s = 1.0 / np.sqrt(H)
    inputs = dict(
        x=rng.standard_normal((B, H), dtype=np.float32),
        W_ih=rng.uniform(-s, s, (3 * H, H)).astype(np.float32),
        W_hh=rng.uniform(-s, s, (3 * H, H)).astype(np.float32),
        b_ih=rng.uniform(-s, s, 3 * H).astype(np.float32),
        b_hh=rng.uniform(-s, s, 3 * H).astype(np.float32),
        seq_len=SEQ,
    )
    out = kernel(**inputs)
    print(out.shape, out.dtype)


# revision 13
# speedup vs baseline: 1.1237x; 1.1237x over previous
"""GRU decoder kernel for Trainium2, 8-core data-parallel.

Problem: h_t recurrence over 512 steps, B=256, H=512 (3 gates).
  xr = relu(x); gi = xr @ W_ih.T + b_ih   (constant over time)
  per step: gh = h @ W_hh.T + b_hh
            r = sigmoid(gi_r + gh_r); z = sigmoid(gi_z + gh_z)
            n = tanh(gi_n + r * gh_n); h = (1-z)*n + z*h
Output: [B, S, H] stacked h_t.

Sharding: pure data parallel, batch/8 = 32 rows per core. Weights replicated.

Per-core design (mapping: psum[batch, gate_out], stationary = h^T tiles):
  - 3 gates computed concurrently via PE col-tiling at tile_position (0,0)/(0,32)/(0,64)
  - gi_r/gi_z and b_hh_n injected into PSUM with identity/ones stationary matmuls
    (b_ih+b_hh folded into gi for r/z on host)
  - hidden dim split in NCHUNK free-dim chunks so the elementwise chain of
    chunk c overlaps the matmuls of chunk c+1 (and PE stays HAM-warm)
  - bf16 state everywhere (DVE 2x mode); f32 only in PSUM and the DRAM output
  - h^T maintained via per-chunk PE transposes (bf16)
"""

import numpy as np

import concourse.bass as bass
from concourse import bacc
import concourse.mybir as mybir
from concourse.tile import TileContext
from concourse.bass_utils import run_bass_kernel_spmd
from concourse.masks import make_identity

B, H, SEQ, NCORES = 256, 512, 512, 8
BL = B // NCORES  # 32 batch rows per core
F32 = mybir.dt.float32
FP16 = mybir.dt.float16
AF = mybir.ActivationFunctionType
NCHUNK = 2
CK = H // NCHUNK  # chunk width in the hidden/free dim


def build_nc(seq_len=SEQ):
    nc = bacc.Bacc("TRN2")
    x_p = nc.declare_dram_parameter("x", [BL, H], F32, isOutput=False)
    wih_p = nc.declare_dram_parameter("W_ihT", [H, 3 * H], F32, isOutput=False)
    whh_p = nc.declare_dram_parameter("W_hhT", [H, 3 * H], F32, isOutput=False)
    bc_p = nc.declare_dram_parameter("b_comb", [1, 3 * H], F32, isOutput=False)
    bn_p = nc.declare_dram_parameter("b_hh_n", [1, H], F32, isOutput=False)
    out_p = nc.declare_dram_parameter("out", [BL, seq_len, H], F32, isOutput=True)

    with TileContext(nc) as tc:
        with (
            tc.tile_pool(name="const", bufs=1) as cpool,
            tc.tile_pool(name="wpool", bufs=1) as wpool,
            tc.tile_pool(name="work", bufs=4) as pool,
            tc.tile_pool(name="hpool", bufs=3) as hpool,
            tc.tile_pool(name="pgates", bufs=2, space="PSUM") as ppool,
            tc.tile_pool(name="ptrans", bufs=2, space="PSUM") as tpool,
        ):
            # ---- constants (bf16 matmul operands) ----
            ident_f = cpool.tile([32, 32], F32)
            make_identity(nc, ident_f)
            ident = cpool.tile([32, 32], FP16)
            nc.vector.tensor_copy(out=ident, in_=ident_f)
            ones1 = cpool.tile([1, 32], FP16)
            nc.vector.memset(ones1, 1.0)
            bhhn_f = cpool.tile([1, H], F32)
            nc.sync.dma_start(out=bhhn_f, in_=bn_p[:, :])
            bhhn = cpool.tile([1, H], FP16)
            nc.vector.tensor_copy(out=bhhn, in_=bhhn_f)
            bcomb_f = cpool.tile([1, 3 * H], F32)
            nc.sync.dma_start(out=bcomb_f, in_=bc_p[:, :])
            bcomb = cpool.tile([1, 3 * H], FP16)
            nc.vector.tensor_copy(out=bcomb, in_=bcomb_f)

            # ---- weights: k-tiles [128, 1536] in bf16 ----
            whh_sb = []
            wih_sb = []
            for j in range(4):
                tf = wpool.tile([128, 3 * H], F32, tag="wstage")
                nc.sync.dma_start(out=tf, in_=whh_p[128 * j : 128 * (j + 1), :])
                tb = wpool.tile([128, 3 * H], FP16, tag=f"whhb{j}")
                nc.vector.tensor_copy(out=tb, in_=tf)
                whh_sb.append(tb)
            for j in range(4):
                tf = wpool.tile([128, 3 * H], F32, tag="wstage")
                nc.sync.dma_start(out=tf, in_=wih_p[128 * j : 128 * (j + 1), :])
                tb = wpool.tile([128, 3 * H], FP16, tag=f"wihb{j}")
                nc.vector.tensor_copy(out=tb, in_=tf)
                wih_sb.append(tb)

            # ---- x -> relu(bf16) -> transpose ----
            x_sb = pool.tile([BL, H], F32, tag="x")
            nc.sync.dma_start(out=x_sb, in_=x_p[:, :])
            xr = pool.tile([BL, H], FP16, tag="xr")
            nc.scalar.activation(out=xr, in_=x_sb, func=AF.Relu)
            psT0 = tpool.tile([128, 128], FP16, tag="pT")
            for j in range(4):
                nc.tensor.transpose(
                    out=psT0[:, 32 * j : 32 * (j + 1)],
                    in_=xr[:, 128 * j : 128 * (j + 1)],
                    identity=ident,
                )
            xrT = pool.tile([128, 128], FP16, tag="xrT")
            nc.vector.tensor_copy(out=xrT, in_=psT0)

            # ---- gi = xr @ W_ihT + b_comb  -> sbuf bf16 [32, 1536] ----
            gi = pool.tile([BL, 3 * H], FP16, tag="gi")
            for g in range(3):
                pg = ppool.tile([128, 512], F32, tag="ps")
                for j in range(4):
                    nc.tensor.matmul(
                        out=pg[0:BL, :],
                        lhsT=xrT[:, 32 * j : 32 * (j + 1)],
                        rhs=wih_sb[j][:, 512 * g : 512 * (g + 1)],
                        start=(j == 0),
                        stop=False,
                    )
                nc.tensor.matmul(
                    out=pg[0:BL, :],
                    lhsT=ones1,
                    rhs=bcomb[:, 512 * g : 512 * (g + 1)],
                    start=False,
                    stop=True,
                )
                nc.scalar.copy(out=gi[:, 512 * g : 512 * (g + 1)], in_=pg[0:BL, :])

            # ---- initial state (h == 0) ----
            hT = pool.tile([128, 128], FP16, tag="hT")
            nc.vector.memset(hT, 0.0)
            h = hpool.tile([BL, H], F32, tag="h")
            nc.vector.memset(h, 0.0)

            # ---- recurrence ----
            for t in range(seq_len):
                ps = ppool.tile([128, 512], F32, tag="ps")
                psT = tpool.tile([128, 128], F32, tag="pT")
                hT_new = pool.tile([128, 128], FP16, tag="hT")
                h_new = hpool.tile([BL, H], F32, tag="h")
                for c in range(NCHUNK):
                    cs = slice(CK * c, CK * (c + 1))
                    # --- matmuls for this free-dim chunk (3 gates col-tiled) ---
                    nc.tensor.matmul(
                        out=ps[0:32, cs], lhsT=ident, rhs=gi[:, CK * c : CK * (c + 1)],
                        start=True, stop=False, tile_position=(0, 0),
                    )
                    nc.tensor.matmul(
                        out=ps[32:64, cs], lhsT=ident,
                        rhs=gi[:, 512 + CK * c : 512 + CK * (c + 1)],
                        start=True, stop=False, tile_position=(0, 32),
                    )
                    nc.tensor.matmul(
                        out=ps[64:96, cs], lhsT=ones1, rhs=bhhn[:, cs],
                        start=True, stop=False, tile_position=(0, 64),
                    )
                    for j in range(4):
                        for g in range(3):
                            nc.tensor.matmul(
                                out=ps[32 * g : 32 * (g + 1), cs],
                                lhsT=hT[:, 32 * j : 32 * (j + 1)],
                                rhs=whh_sb[j][:, 512 * g + CK * c : 512 * g + CK * (c + 1)],
                                start=False,
                                stop=(j == 3),
                                tile_position=(0, 32 * g),
                            )
                    # --- gates for this chunk ---
                    r = pool.tile([32, CK], FP16, tag="r")
                    nc.scalar.activation(out=r, in_=ps[0:32, cs], func=AF.Sigmoid)
                    z = pool.tile([32, CK], FP16, tag="z")
                    nc.scalar.activation(out=z, in_=ps[32:64, cs], func=AF.Sigmoid)
                    m = pool.tile([32, CK], FP16, tag="m")
                    nc.vector.tensor_mul(out=m, in0=r, in1=ps[64:96, cs])
                    a = pool.tile([32, CK], FP16, tag="a")
                    nc.vector.tensor_add(out=a, in0=m, in1=gi[:, 1024 + CK * c : 1024 + CK * (c + 1)])
                    n_ = pool.tile([32, CK], FP16, tag="n")
                    nc.scalar.activation(out=n_, in_=a, func=AF.Tanh)
                    # h_new = n + z*(h - n); h kept f32 (it is the DRAM output)
                    d = pool.tile([32, CK], FP16, tag="d")
                    nc.vector.tensor_sub(out=d, in0=h[:, cs], in1=n_)
                    t2 = pool.tile([32, CK], FP16, tag="t2")
                    nc.vector.tensor_mul(out=t2, in0=z, in1=d)
                    nc.vector.tensor_add(out=h_new[:, cs], in0=n_, in1=t2)
                    # transpose this chunk for the next step's stationaries
                    if t < seq_len - 1:
                        for jj in range(2 * c, 2 * c + 2):
                            nc.tensor.transpose(
                                out=psT[:, 32 * jj : 32 * (jj + 1)],
                                in_=h_new[:, 128 * jj : 128 * (jj + 1)],
                                identity=ident_f,
                            )
                        nc.vector.tensor_copy(
                            out=hT_new[:, 64 * c : 64 * (c + 1)],
                            in_=psT[:, 64 * c : 64 * (c + 1)],
                        )
                nc.sync.dma_start(out=out_p[:, t, :], in_=h_new)
                h = h_new
                hT = hT_new
    nc.finalize()
    return nc


def kernel(x, W_ih, W_hh, b_ih, b_hh, seq_len):
    x = np.asarray(x, dtype=np.float32)
    W_ih = np.asarray(W_ih, dtype=np.float32)
    W_hh = np.asarray(W_hh, dtype=np.float32)
    b_ih = np.asarray(b_ih, dtype=np.float32)
    b_hh = np.asarray(b_hh, dtype=np.float32)
    seq_len = int(seq_len)

    W_ihT = np.ascontiguousarray(W_ih.T)
    W_hhT = np.ascontiguousarray(W_hh.T)
    b_comb = np.concatenate([b_ih[: 2 * H] + b_hh[: 2 * H], b_ih[2 * H :]]).reshape(1, -1)
    b_hh_n = np.ascontiguousarray(b_hh[2 * H :]).reshape(1, -1)

    nc = build_nc(seq_len=seq_len)
    in_maps = [
        {
            "x": np.ascontiguousarray(x[BL * i : BL * (i + 1)]),
            "W_ihT": W_ihT,
            "W_hhT": W_hhT,
            "b_comb": b_comb,
            "b_hh_n": b_hh_n,
        }
        for i in range(NCORES)
    ]
    res = run_bass_kernel_spmd(nc, in_maps, core_ids=list(range(NCORES)))
    outs = [np.asarray(res.results[i]["out"], dtype=np.float32) for i in range(NCORES)]
    return np.concatenate(outs, axis=0)


if __name__ == "__main__":
    rng = np.random.default_rng(0)
    s = 1.0 / np.sqrt(H)
    inputs = dict(
        x=rng.standard_normal((B, H), dtype=np.float32),
        W_ih=rng.uniform(-s, s, (3 * H, H)).astype(np.float32),
        W_hh=rng.uniform(-s, s, (3 * H, H)).astype(np.float32),
        b_ih=rng.uniform(-s, s, 3 * H).astype(np.float32),
        b_hh=rng.uniform(-s, s, 3 * H).astype(np.float32),
        seq_len=SEQ,
    )
    out = kernel(**inputs)
    print(out.shape, out.dtype)


# revision 17
# speedup vs baseline: 1.3097x; 1.1655x over previous
"""GRU decoder kernel for Trainium2, 8-core data-parallel.

Problem: h_t recurrence over 512 steps, B=256, H=512 (3 gates).
  xr = relu(x); gi = xr @ W_ih.T + b_ih   (constant over time)
  per step: gh = h @ W_hh.T + b_hh
            r = sigmoid(gi_r + gh_r); z = sigmoid(gi_z + gh_z)
            n = tanh(gi_n + r * gh_n); h = (1-z)*n + z*h
Output: [B, S, H] stacked h_t.

Sharding: pure data parallel, batch/8 = 32 rows per core. Weights replicated.

Per-core design (mapping: psum[batch, gate_out], stationary = h^T tiles):
  - 3 gates computed concurrently via PE col-tiling at tile_position (0,0)/(0,32)/(0,64)
  - gi_r/gi_z and b_hh_n injected into PSUM with identity/ones stationary matmuls
    (b_ih+b_hh folded into gi for r/z on host)
  - hidden dim split in NCHUNK free-dim chunks so the elementwise chain of
    chunk c overlaps the matmuls of chunk c+1 (and PE stays HAM-warm)
  - bf16 state everywhere (DVE 2x mode); f32 only in PSUM and the DRAM output
  - h^T maintained via per-chunk PE transposes (bf16)
"""

import numpy as np

import concourse.bass as bass
from concourse import bacc
import concourse.mybir as mybir
from concourse.tile import TileContext
from concourse.bass_utils import run_bass_kernel_spmd
from concourse.masks import make_identity

B, H, SEQ, NCORES = 256, 512, 512, 8
BL = B // NCORES  # 32 batch rows per core
F32 = mybir.dt.float32
FP16 = mybir.dt.float16
AF = mybir.ActivationFunctionType
NCHUNK = 2
CK = H // NCHUNK  # chunk width in the hidden/free dim


def build_nc(seq_len=SEQ):
    nc = bacc.Bacc("TRN2")
    x_p = nc.declare_dram_parameter("x", [BL, H], F32, isOutput=False)
    wih_p = nc.declare_dram_parameter("W_ihT", [H, 3 * H], F32, isOutput=False)
    whh_p = nc.declare_dram_parameter("W_hhT", [H, 3 * H], F32, isOutput=False)
    bc_p = nc.declare_dram_parameter("b_comb", [1, 3 * H], F32, isOutput=False)
    bn_p = nc.declare_dram_parameter("b_hh_n", [1, H], F32, isOutput=False)
    out_p = nc.declare_dram_parameter("out", [BL, seq_len, H], F32, isOutput=True)

    with TileContext(nc) as tc:
        with (
            tc.tile_pool(name="const", bufs=1) as cpool,
            tc.tile_pool(name="wpool", bufs=1) as wpool,
            tc.tile_pool(name="work", bufs=4) as pool,
            tc.tile_pool(name="hpool", bufs=3) as hpool,
            tc.tile_pool(name="pgates", bufs=2, space="PSUM") as ppool,
            tc.tile_pool(name="ptrans", bufs=2, space="PSUM") as tpool,
        ):
            # ---- constants (bf16 matmul operands) ----
            ident_f = cpool.tile([32, 32], F32)
            make_identity(nc, ident_f)
            ident = cpool.tile([32, 32], FP16)
            nc.vector.tensor_copy(out=ident, in_=ident_f)
            ones1 = cpool.tile([1, 32], FP16)
            nc.vector.memset(ones1, 1.0)
            bhhn_f = cpool.tile([1, H], F32)
            nc.sync.dma_start(out=bhhn_f, in_=bn_p[:, :])
            bhhn = cpool.tile([1, H], FP16)
            nc.vector.tensor_copy(out=bhhn, in_=bhhn_f)
            bcomb_f = cpool.tile([1, 3 * H], F32)
            nc.sync.dma_start(out=bcomb_f, in_=bc_p[:, :])
            bcomb = cpool.tile([1, 3 * H], FP16)
            nc.vector.tensor_copy(out=bcomb, in_=bcomb_f)

            # ---- weights: k-tiles [128, 1536] in bf16 ----
            whh_sb = []
            wih_sb = []
            for j in range(4):
                tf = wpool.tile([128, 3 * H], F32, tag="wstage")
                nc.sync.dma_start(out=tf, in_=whh_p[128 * j : 128 * (j + 1), :])
                tb = wpool.tile([128, 3 * H], FP16, tag=f"whhb{j}")
                nc.vector.tensor_copy(out=tb, in_=tf)
                whh_sb.append(tb)
            for j in range(4):
                tf = wpool.tile([128, 3 * H], F32, tag="wstage")
                nc.sync.dma_start(out=tf, in_=wih_p[128 * j : 128 * (j + 1), :])
                tb = wpool.tile([128, 3 * H], FP16, tag=f"wihb{j}")
                nc.vector.tensor_copy(out=tb, in_=tf)
                wih_sb.append(tb)

            # ---- x -> relu(bf16) -> transpose ----
            x_sb = pool.tile([BL, H], F32, tag="x")
            nc.sync.dma_start(out=x_sb, in_=x_p[:, :])
            xr = pool.tile([BL, H], FP16, tag="xr")
            nc.scalar.activation(out=xr, in_=x_sb, func=AF.Relu)
            psT0 = tpool.tile([128, 128], FP16, tag="pT0")
            for j in range(4):
                nc.tensor.transpose(
                    out=psT0[:, 32 * j : 32 * (j + 1)],
                    in_=xr[:, 128 * j : 128 * (j + 1)],
                    identity=ident,
                )
            xrT = pool.tile([128, 128], FP16, tag="xrT")
            nc.vector.tensor_copy(out=xrT, in_=psT0)

            # ---- gi = xr @ W_ihT + b_comb  -> sbuf bf16 [32, 1536] ----
            gi = pool.tile([BL, 3 * H], FP16, tag="gi")
            for g in range(3):
                pg = ppool.tile([128, 512], F32, tag="ps0")
                for j in range(4):
                    nc.tensor.matmul(
                        out=pg[0:BL, :],
                        lhsT=xrT[:, 32 * j : 32 * (j + 1)],
                        rhs=wih_sb[j][:, 512 * g : 512 * (g + 1)],
                        start=(j == 0),
                        stop=False,
                    )
                nc.tensor.matmul(
                    out=pg[0:BL, :],
                    lhsT=ones1,
                    rhs=bcomb[:, 512 * g : 512 * (g + 1)],
                    start=False,
                    stop=True,
                )
                nc.scalar.copy(out=gi[:, 512 * g : 512 * (g + 1)], in_=pg[0:BL, :])

            # ---- PE warm-up: ~6us of dense matmuls lifts HAM to K=8/8 ----
            for w in range(24):
                pw = ppool.tile([128, 512], F32, tag=f"ps{w % 2}")
                nc.tensor.matmul(
                    out=pw[0:32, :], lhsT=ident, rhs=gi[:, 0:512],
                    start=True, stop=True,
                )

            # ---- initial state (h == 0) ----
            hT0 = pool.tile([128, 64], FP16, tag="hT0")  # k-tiles 0,1
            nc.vector.memset(hT0, 0.0)
            hT1 = pool.tile([128, 64], FP16, tag="hT1")  # k-tiles 2,3
            nc.vector.memset(hT1, 0.0)
            halves = []
            for c in range(2):
                hc = hpool.tile([BL, CK], F32, tag=f"h{c}")
                nc.vector.memset(hc, 0.0)
                halves.append(hc)

            # ---- recurrence ----
            # pass order is k-pair-major: passes over k-tiles {0,1} only need
            # hT0 (chunk 0 of h_{t-1}); {2,3} need hT1.  The chunk-1 gate
            # chain is emitted FIRST: its transposes feed hT1 which gates the
            # next step's critical j23 passes.
            for t in range(seq_len):
                ps_c0 = ppool.tile([128, CK], F32, tag="ps0", name=f"psc0_{t}")
                ps_c1 = ppool.tile([128, CK], F32, tag="ps1", name=f"psc1_{t}")
                psc = [ps_c0, ps_c1]
                for c in range(2):
                    cs = slice(CK * c, CK * (c + 1))
                    nc.tensor.matmul(
                        out=psc[c][0:32, :], lhsT=ident, rhs=gi[:, CK * c : CK * (c + 1)],
                        start=True, stop=False, tile_position=(0, 0),
                    )
                    nc.tensor.matmul(
                        out=psc[c][32:64, :], lhsT=ident,
                        rhs=gi[:, 512 + CK * c : 512 + CK * (c + 1)],
                        start=True, stop=False, tile_position=(0, 32),
                    )
                    nc.tensor.matmul(
                        out=psc[c][64:96, :], lhsT=ones1, rhs=bhhn[:, cs],
                        start=True, stop=False, tile_position=(0, 64),
                    )
                for j in range(4):
                    hT_j = hT0[:, 32 * j : 32 * (j + 1)] if j < 2 else hT1[:, 32 * (j - 2) : 32 * (j - 1)]
                    for c in range(2):
                        for g in range(3):
                            nc.tensor.matmul(
                                out=psc[c][32 * g : 32 * (g + 1), :],
                                lhsT=hT_j,
                                rhs=whh_sb[j][:, 512 * g + CK * c : 512 * g + CK * (c + 1)],
                                start=False,
                                stop=(j == 3),
                                tile_position=(0, 32 * g),
                            )
                new_halves = [None, None]
                new_hT = [None, None]
                for c in (1, 0):  # chunk 1 first: it gates the next step's j23
                    ps = psc[c]
                    r = pool.tile([32, CK], FP16, tag=f"r{c}")
                    nc.scalar.activation(out=r, in_=ps[0:32, :], func=AF.Sigmoid)
                    z = pool.tile([32, CK], FP16, tag=f"z{c}")
                    nc.scalar.activation(out=z, in_=ps[32:64, :], func=AF.Sigmoid)
                    m = pool.tile([32, CK], FP16, tag=f"m{c}")
                    nc.vector.tensor_mul(out=m, in0=r, in1=ps[64:96, :])
                    a = pool.tile([32, CK], FP16, tag=f"a{c}")
                    nc.vector.tensor_add(out=a, in0=m, in1=gi[:, 1024 + CK * c : 1024 + CK * (c + 1)])
                    n_ = pool.tile([32, CK], FP16, tag=f"n{c}")
                    nc.scalar.activation(out=n_, in_=a, func=AF.Tanh)
                    # h_new = n + z*(h - n)
                    d = pool.tile([32, CK], FP16, tag=f"d{c}")
                    nc.vector.tensor_sub(out=d, in0=halves[c], in1=n_)
                    t2 = pool.tile([32, CK], FP16, tag=f"t2{c}")
                    nc.vector.tensor_mul(out=t2, in0=z, in1=d)
                    hc = hpool.tile([BL, CK], F32, tag=f"h{c}")
                    nc.vector.tensor_add(out=hc, in0=n_, in1=t2)
                    new_halves[c] = hc
                    if t < seq_len - 1:
                        psT = tpool.tile([128, 64], F32, tag=f"pT{c}")
                        for jj in range(2):
                            nc.tensor.transpose(
                                out=psT[:, 32 * jj : 32 * (jj + 1)],
                                in_=hc[:, 128 * jj : 128 * (jj + 1)],
                                identity=ident_f,
                            )
                        hTc = pool.tile([128, 64], FP16, tag=f"hT{c}")
                        nc.vector.tensor_copy(out=hTc, in_=psT)
                        new_hT[c] = hTc
                for c in range(2):
                    nc.sync.dma_start(
                        out=out_p[:, t, CK * c : CK * (c + 1)], in_=new_halves[c]
                    )
                halves = new_halves
                hT0, hT1 = new_hT
    nc.finalize()
    return nc


def kernel(x, W_ih, W_hh, b_ih, b_hh, seq_len):
    x = np.asarray(x, dtype=np.float32)
    W_ih = np.asarray(W_ih, dtype=np.float32)
    W_hh = np.asarray(W_hh, dtype=np.float32)
    b_ih = np.asarray(b_ih, dtype=np.float32)
    b_hh = np.asarray(b_hh, dtype=np.float32)
    seq_len = int(seq_len)

    W_ihT = np.ascontiguousarray(W_ih.T)
    W_hhT = np.ascontiguousarray(W_hh.T)
    b_comb = np.concatenate([b_ih[: 2 * H] + b_hh[: 2 * H], b_ih[2 * H :]]).reshape(1, -1)
    b_hh_n = np.ascontiguousarray(b_hh[2 * H :]).reshape(1, -1)

    nc = build_nc(seq_len=seq_len)
    in_maps = [
        {
            "x": np.ascontiguousarray(x[BL * i : BL * (i + 1)]),
            "W_ihT": W_ihT,
            "W_hhT": W_hhT,
            "b_comb": b_comb,
            "b_hh_n": b_hh_n,
        }
        for i in range(NCORES)
    ]
    res = run_bass_kernel_spmd(nc, in_maps, core_ids=list(range(NCORES)))
    outs = [np.asarray(res.results[i]["out"], dtype=np.float32) for i in range(NCORES)]
    return np.concatenate(outs, axis=0)


if __name__ == "__main__":
    rng = np.random.default_rng(0)
    s = 1.0 / np.sqrt(H)
    inputs = dict(
        x=rng.standard_normal((B, H), dtype=np.float32),
        W_ih=rng.uniform(-s, s, (3 * H, H)).astype(np.float32),
        W_hh=rng.uniform(-s, s, (3 * H, H)).astype(np.float32),
        b_ih=rng.uniform(-s, s, 3 * H).astype(np.float32),
        b_hh=rng.uniform(-s, s, 3 * H).astype(np.float32),
        seq_len=SEQ,
    )
    out = kernel(**inputs)
    print(out.shape, out.dtype)
